# revision 9
# baseline (speedup 1.0000x reference)
"""Cross-attention layer (vision<->text) on 8 Trainium2 NeuronCores.

Problem: B=16, Sv=St=1024, D=1024, fp32.
  q = vision @ Wq.T + bq            [B,Sv,D]
  k = text   @ Wk.T + bk            [B,St,D]
  v = text   @ Wv.T + bv            [B,St,D]
  scores = q @ k.T / sqrt(D)        [B,Sv,St]
  attn = softmax(scores, -1)
  cross_vision = attn @ v           [B,Sv,D]
  cross_text   = attn.T @ vision    [B,St,D]

Sharding: pure data-parallel over batch, 2 items per core, no collectives.

Design (v2 — PE-stream-first):
  - Host stages activations in BOTH orientations as bf16 (visT/txtT [d,s]
    and vis natural [s,d]) plus weights pre-tiled for direct stationary
    loads.  This removes all on-chip PE transposes of the inputs (256 per
    core in v1) and the staging copies that came with them.
  - All matmul operands are bf16 (PSUM accumulation stays fp32).  At
    N=512 the PE streams 1 row/cycle for bf16 and fp32r alike, so this
    costs nothing on the matmul stream but makes everything resident in
    SBUF (181KB/partition), halves DMA, speeds the E-transposes
    (1.0 vs 1.5 cyc/row) and lets LDWEIGHTS use fast-weight-load.
  - Per item: projK -> projV (V computed directly in [t,d] layout with
    text tiles stationary: no transposes) -> projQ -> F (scores/exp/
    E-transpose/cross_vision, software-pipelined one s-tile ahead) ->
    H (cross_text, 16 sequential single-bank PSUM groups).
  - The 1/sqrt(D) is folded into wqt/bq on the host.  bv is added on the
    host after gather (attn rows sum to 1, so attn @ (v0+bv) = attn@v0
    + bv, exact).
  - Weights are loaded once and stay resident across both items; the
    next item's activations prefetch during the current item's F/H so
    the PE never waits at an item boundary (keeps the PE HAM throttle
    at 8/8 — every >3.4us PE idle gap re-throttles the clock to 1.2GHz
    for ~10us).
"""

import sys

import numpy as np

if "/opt/trn_rl_repo" not in sys.path:
    sys.path.insert(0, "/opt/trn_rl_repo")

import ml_dtypes

import concourse.bass as bass
import concourse.tile as tile
from concourse import bacc
from concourse import mybir

P = 128
B, SEQ, DIM = 16, 1024, 1024
N_CORES = 8
BPC = B // N_CORES  # batch items per core
NT = DIM // P  # 8 tiles of 128 along d/e
F32 = mybir.dt.float32
BF16 = mybir.dt.bfloat16
AF = mybir.ActivationFunctionType
H = 512  # half of a seq dim / PSUM-bank-sized chunk
BF_NP = ml_dtypes.bfloat16


def _emit_item(nc, b, tens, pools, cur, nxt):
    """Emit one batch item.  `cur` holds this item's already-loading
    activation tiles (txtT/visT/visn); prefetch tiles for item b+1 are
    allocated here mid-item and returned via `nxt`."""
    (txtT_d, visT_d, visn_d, cv_d, ct_d) = tens["dram"]
    (wq_sb, wk_sb, wv_sb, bq_sb, bk_sb, ident) = tens["wt"]
    (p_txtT, p_visT, p_visn, p_kt, p_qt, p_vv, p_esb, p_etb, p_cvs, p_cts,
     p_rp, p_rv, pp_t, pp_mm) = pools

    txtT, visT, visn = cur["txtT"], cur["visT"], cur["visn"]
    kt = p_kt.tile([P, NT, SEQ], BF16, name="kt", tag="kt")
    qt = p_qt.tile([P, NT, SEQ], BF16, name="qt", tag="qt")
    vv = p_vv.tile([P, NT, SEQ], BF16, name="vv", tag="vv")
    e_sb = p_esb.tile([P, NT, SEQ], BF16, name="e_sb", tag="esb")

    def nxt_loads(phase):
        """Prefetch item b+1 activations once item b is done reading."""
        if b + 1 >= BPC:
            return
        if phase == "projQ":
            # txtT(b) is dead after projV(b); same buffer, sync queue.
            t2 = p_txtT.tile([P, NT, SEQ], BF16, name="txtT", tag="txtT")
            for l in range(NT):
                nc.sync.dma_start(out=t2[:, l, :], in_=txtT_d[b + 1, :, l, :])
            nxt["txtT"] = t2
        elif phase == "F":
            # visT(b) is dead after projQ(b).
            v2 = p_visT.tile([P, NT, SEQ], BF16, name="visT", tag="visT")
            for l in range(NT):
                nc.sync.dma_start(out=v2[:, l, :], in_=visT_d[b + 1, :, l, :])
            nxt["visT"] = v2
            # vis_nat has bufs=2, so this never blocks the gpsimd queue.
            v3 = p_visn.tile([P, NT, SEQ], BF16, name="visn", tag="visn")
            nc.gpsimd.dma_start(out=v3, in_=visn_d[b + 1])
            nxt["visn"] = v3

    # ---- projK: kt[e, t] = sum_do wkt[do,:,e].T @ txtT[do,:,t] (+bk) ----
    def proj(w_sb, bias_col, xT, out_sb):
        for eo in range(NT):
            pss = [pp_mm.tile([P, H], F32, name=f"ps_p{i}", tag="mm")
                   for i in range(2)]
            for do in range(NT):
                for sh in range(2):
                    nc.tensor.matmul(pss[sh], w_sb[:, eo, do, :],
                                     xT[:, do, sh * H:(sh + 1) * H],
                                     start=(do == 0), stop=(do == NT - 1))
            for sh in range(2):
                dst = out_sb[:, eo, sh * H:(sh + 1) * H]
                if sh == 0:
                    nc.vector.tensor_scalar_add(dst, pss[sh],
                                                scalar1=bias_col[:, eo:eo + 1])
                else:
                    nc.scalar.add(dst, pss[sh], add=bias_col[:, eo:eo + 1])

    proj(wk_sb, bk_sb, txtT, kt)

    # ---- projV: vv[t, d] = sum_do txtT[do,:,t-block].T @ wvt[do,:,d] ----
    for tb in range(NT):
        pss = [pp_mm.tile([P, H], F32, name=f"ps_v{i}", tag="mm")
               for i in range(2)]
        for do in range(NT):
            for dh in range(2):
                nc.tensor.matmul(pss[dh], txtT[:, do, tb * P:(tb + 1) * P],
                                 wv_sb[:, do, dh * H:(dh + 1) * H],
                                 start=(do == 0), stop=(do == NT - 1))
        for dh in range(2):
            dst = vv[:, tb, dh * H:(dh + 1) * H]
            if dh == 0:
                nc.vector.tensor_copy(dst, pss[dh])
            else:
                nc.scalar.copy(dst, pss[dh])

    # ---- projQ ----
    nxt_loads("projQ")
    proj(wq_sb, bq_sb, visT, qt)

    # ---- phase F: scores, softmax, cross_vision (per s-tile) ----
    # Software-pipelined: the scores matmuls of s-tile so+1 are emitted
    # between exp(so) (ACT) and the E-transposes that consume it, so the
    # in-order PE never waits on the ACT engine.
    nxt_loads("F")
    rinv = p_rv.tile([P, NT], F32, name="rinv", tag="rinv")
    rps = {}

    def scores_stile(so):
        rp = p_rp.tile([P, 2], F32, name="rp", tag="rp")
        pss = [pp_mm.tile([P, H], F32, name=f"ps_s{i}", tag="mm")
               for i in range(2)]
        for eo in range(NT):
            for th in range(2):
                nc.tensor.matmul(pss[th], qt[:, eo, so * P:(so + 1) * P],
                                 kt[:, eo, th * H:(th + 1) * H],
                                 start=(eo == 0), stop=(eo == NT - 1))
        for th in range(2):
            nc.scalar.activation(out=e_sb[:, so, th * H:(th + 1) * H],
                                 in_=pss[th], func=AF.Exp,
                                 accum_out=rp[:, th:th + 1])
        rps[so] = rp

    def cross_vision(so, etb):
        # cross_vision[s-tile] = rinv * (ET.T @ Vv)
        cvs = p_cvs.tile([P, DIM], F32, name="cvs", tag="cvs")
        pcv = [pp_mm.tile([P, H], F32, name=f"ps_cv{i}", tag="mm")
               for i in range(2)]
        for tt in range(NT):
            for dh in range(2):
                nc.tensor.matmul(pcv[dh], etb[:, tt, :],
                                 vv[:, tt, dh * H:(dh + 1) * H],
                                 start=(tt == 0), stop=(tt == NT - 1))
        for dh in range(2):
            nc.scalar.mul(cvs[:, dh * H:(dh + 1) * H], pcv[dh],
                          mul=rinv[:, so:so + 1])
        nc.scalar.dma_start(out=cv_d[b, so * P:(so + 1) * P, :], in_=cvs)

    # Pipelined two deep: iteration `so` emits scores(so+1), the
    # E-transposes of so, and cross_vision(so-1) — so the etb copies that
    # CV consumes are always a full iteration old and the PE never waits
    # on the Vector engine.
    scores_stile(0)
    etbs = {}
    for so in range(NT):
        if so + 1 < NT:
            scores_stile(so + 1)
        rp = rps.pop(so)
        rsum = p_rp.tile([P, 1], F32, name="rsum", tag="rsum")
        nc.vector.tensor_add(rsum, rp[:, 0:1], rp[:, 1:2])
        nc.vector.reciprocal(rinv[:, so:so + 1], rsum)

        # ET blocks for this s-tile (transpose the *unnormalized* E row-block)
        etb = p_etb.tile([P, NT, P], BF16, name="etb", tag="etb")
        for tg in range(2):
            tp4 = pp_t.tile([P, 4, P], BF16, name="tp4e", tag="tp4")
            for j in range(4):
                tt = tg * 4 + j
                nc.tensor.matmul(tp4[:, j, :], e_sb[:, so, tt * P:(tt + 1) * P],
                                 ident, is_transpose=True,
                                 start=(j == 0), stop=(j == 3),
                                 skip_group_check=True)
            nc.vector.tensor_copy(etb[:, tg * 4:(tg + 1) * 4, :], tp4)
        etbs[so] = etb

        # normalize this E row-block in place (for cross_text later)
        nc.vector.tensor_scalar_mul(e_sb[:, so, :], e_sb[:, so, :],
                                    scalar1=rinv[:, so:so + 1])

        if so - 1 >= 0:
            cross_vision(so - 1, etbs.pop(so - 1))
    cross_vision(NT - 1, etbs.pop(NT - 1))

    # ---- phase H: cross_text = E'.T @ vis (E' already rinv-scaled) ----
    # 16 sequential single-bank PSUM groups; vis natural is SBUF-resident
    # so the 128 matmuls stream back-to-back with no DMA dependence.
    for dh in range(2):
        for tt in range(NT):
            ps = pp_mm.tile([P, H], F32, name="ps_ct", tag="mm")
            for so in range(NT):
                nc.tensor.matmul(ps, e_sb[:, so, tt * P:(tt + 1) * P],
                                 visn[:, so, dh * H:(dh + 1) * H],
                                 start=(so == 0), stop=(so == NT - 1))
            cts = p_cts.tile([P, H], F32, name="cts", tag="cts")
            if tt % 2 == 0:
                nc.vector.tensor_copy(cts, ps)
            else:
                nc.scalar.copy(cts, ps)
            nc.gpsimd.dma_start(
                out=ct_d[b, tt * P:(tt + 1) * P, dh * H:(dh + 1) * H], in_=cts)


def build_nc():
    nc = bacc.Bacc("TRN2", target_bir_lowering=False, debug=False,
                   num_devices=N_CORES)
    txtT_d = nc.dram_tensor("txtT", [BPC, P, NT, SEQ], BF16,
                            kind="ExternalInput").ap()
    visT_d = nc.dram_tensor("visT", [BPC, P, NT, SEQ], BF16,
                            kind="ExternalInput").ap()
    visn_d = nc.dram_tensor("visn", [BPC, P, NT, SEQ], BF16,
                            kind="ExternalInput").ap()
    wq_d = nc.dram_tensor("wq", [P, NT, NT, P], BF16, kind="ExternalInput").ap()
    wk_d = nc.dram_tensor("wk", [P, NT, NT, P], BF16, kind="ExternalInput").ap()
    wv_d = nc.dram_tensor("wv", [P, NT, SEQ], BF16, kind="ExternalInput").ap()
    bq_d = nc.dram_tensor("bq", [P, NT], F32, kind="ExternalInput").ap()
    bk_d = nc.dram_tensor("bk", [P, NT], F32, kind="ExternalInput").ap()
    id_d = nc.dram_tensor("ident128", [P, P], BF16, kind="ExternalInput").ap()
    cv_d = nc.dram_tensor("cross_vision", [BPC, SEQ, DIM], F32,
                          kind="ExternalOutput").ap()
    ct_d = nc.dram_tensor("cross_text", [BPC, SEQ, DIM], F32,
                          kind="ExternalOutput").ap()

    with tile.TileContext(nc) as tc:
        import contextlib
        with contextlib.ExitStack() as ctx:
            def sp(name, bufs):
                return ctx.enter_context(tc.tile_pool(name=name, bufs=bufs))

            p_wt = sp("wt", 1)
            p_txtT = sp("txtT", 1)
            p_visT = sp("visT", 1)
            p_visn = sp("visn", 2)
            p_kt = sp("kt", 1)
            p_qt = sp("qt", 1)
            p_vv = sp("vv", 1)
            p_esb = sp("esb", 1)
            p_etb = sp("etb", 2)
            p_cvs = sp("cvs", 2)
            p_cts = sp("cts", 4)
            p_rp = sp("rp", 4)
            p_rv = sp("rv", 2)
            pp_t = ctx.enter_context(
                tc.tile_pool(name="pp_t", bufs=2, space=bass.MemorySpace.PSUM))
            pp_mm = ctx.enter_context(
                tc.tile_pool(name="pp_mm", bufs=6, space=bass.MemorySpace.PSUM))

            # small constants first on sync (needed at first proj evac)
            ident = p_wt.tile([P, P], BF16, name="ident")
            nc.sync.dma_start(out=ident, in_=id_d)
            bq_sb = p_wt.tile([P, NT], F32, name="bq_sb")
            nc.sync.dma_start(out=bq_sb, in_=bq_d)
            bk_sb = p_wt.tile([P, NT], F32, name="bk_sb")
            nc.sync.dma_start(out=bk_sb, in_=bk_d)

            # resident weights on gpsimd, chunked by eo (projK's first PSUM
            # group needs only the eo=0 chunk), in use order (K,V,Q)
            wk_sb = p_wt.tile([P, NT, NT, P], BF16, name="wk_sb")
            for eo in range(NT):
                nc.gpsimd.dma_start(out=wk_sb[:, eo, :, :], in_=wk_d[:, eo, :, :])
            wv_sb = p_wt.tile([P, NT, SEQ], BF16, name="wv_sb")
            for do in range(NT):
                nc.gpsimd.dma_start(out=wv_sb[:, do, :], in_=wv_d[:, do, :])
            wq_sb = p_wt.tile([P, NT, NT, P], BF16, name="wq_sb")
            for eo in range(NT):
                nc.gpsimd.dma_start(out=wq_sb[:, eo, :, :], in_=wq_d[:, eo, :, :])

            tens = {
                "dram": (txtT_d, visT_d, visn_d, cv_d, ct_d),
                "wt": (wq_sb, wk_sb, wv_sb, bq_sb, bk_sb, ident),
            }
            pools = (p_txtT, p_visT, p_visn, p_kt, p_qt, p_vv, p_esb, p_etb,
                     p_cvs, p_cts, p_rp, p_rv, pp_t, pp_mm)

            # item-0 activation loads (chunked so the first matmuls can
            # start before the full tensors land): txtT split across the
            # sync and scalar queues (it gates projK's first PSUM group),
            # visT behind it on sync, vis natural on gpsimd behind the
            # weights.
            cur = {}
            cur["txtT"] = p_txtT.tile([P, NT, SEQ], BF16, name="txtT",
                                      tag="txtT")
            for l in range(NT):
                eng = nc.sync if l % 2 == 0 else nc.scalar
                eng.dma_start(out=cur["txtT"][:, l, :],
                              in_=txtT_d[0, :, l, :])
            cur["visT"] = p_visT.tile([P, NT, SEQ], BF16, name="visT",
                                      tag="visT")
            for l in range(NT):
                nc.sync.dma_start(out=cur["visT"][:, l, :],
                                  in_=visT_d[0, :, l, :])
            cur["visn"] = p_visn.tile([P, NT, SEQ], BF16, name="visn",
                                      tag="visn")
            nc.gpsimd.dma_start(out=cur["visn"], in_=visn_d[0])

            for b in range(BPC):
                nxt = {}
                _emit_item(nc, b, tens, pools, cur, nxt)
                cur = nxt
    nc.compile()
    return nc


_NC_CACHE = None


def _get_nc():
    global _NC_CACHE
    if _NC_CACHE is None:
        _NC_CACHE = build_nc()
    return _NC_CACHE


def make_in_maps(vision_repr, text_repr, Wq, bq, Wk, bk, Wv, bv):
    s = 1.0 / np.sqrt(np.float32(DIM))

    def wtile(w, scale=None):
        wt = np.asarray(w, np.float32).T
        if scale is not None:
            wt = wt * scale
        # [d, e] -> [di, eo, do, ei]
        return np.ascontiguousarray(
            wt.reshape(NT, P, NT, P).transpose(1, 2, 0, 3)).astype(BF_NP)

    wq_t = wtile(Wq, s)
    wk_t = wtile(Wk)
    wv_t = np.ascontiguousarray(
        np.asarray(Wv, np.float32).T.reshape(NT, P, SEQ).transpose(1, 0, 2)
    ).astype(BF_NP)
    bq_s = np.ascontiguousarray(
        (np.asarray(bq, np.float32) * s).reshape(NT, P).T)
    bk_s = np.ascontiguousarray(np.asarray(bk, np.float32).reshape(NT, P).T)

    vis = np.asarray(vision_repr, np.float32)
    txt = np.asarray(text_repr, np.float32)
    # [b, s, d] -> transposed [b, di, l, s] and natural [b, si, so, d]
    visT = np.ascontiguousarray(
        vis.transpose(0, 2, 1).reshape(B, NT, P, SEQ).transpose(0, 2, 1, 3)
    ).astype(BF_NP)
    txtT = np.ascontiguousarray(
        txt.transpose(0, 2, 1).reshape(B, NT, P, SEQ).transpose(0, 2, 1, 3)
    ).astype(BF_NP)
    visn = np.ascontiguousarray(
        vis.reshape(B, NT, P, DIM).transpose(0, 2, 1, 3)).astype(BF_NP)

    ident = np.eye(P, dtype=BF_NP)
    in_maps = []
    for c in range(N_CORES):
        sl = slice(c * BPC, (c + 1) * BPC)
        in_maps.append({
            "txtT": txtT[sl], "visT": visT[sl], "visn": visn[sl],
            "wq": wq_t, "wk": wk_t, "wv": wv_t,
            "bq": bq_s, "bk": bk_s,
            "ident128": ident,
        })
    return in_maps


def kernel(vision_repr, text_repr, Wq, bq, Wk, bk, Wv, bv):
    from concourse.bass_utils import run_bass_kernel_spmd

    nc = _get_nc()
    in_maps = make_in_maps(vision_repr, text_repr, Wq, bq, Wk, bk, Wv, bv)
    res = run_bass_kernel_spmd(nc, in_maps, list(range(N_CORES))).results
    cv = np.concatenate([r_["cross_vision"] for r_ in res], axis=0)
    ct = np.concatenate([r_["cross_text"] for r_ in res], axis=0)
    cv = cv + np.asarray(bv, np.float32)[None, None, :]
    return cv, ct


# revision 10
# speedup vs baseline: 1.1657x; 1.1657x over previous
"""Cross-attention layer (vision<->text) on 8 Trainium2 NeuronCores.

Problem: B=16, Sv=St=1024, D=1024, fp32.
  q = vision @ Wq.T + bq            [B,Sv,D]
  k = text   @ Wk.T + bk            [B,St,D]
  v = text   @ Wv.T + bv            [B,St,D]
  scores = q @ k.T / sqrt(D)        [B,Sv,St]
  attn = softmax(scores, -1)
  cross_vision = attn @ v           [B,Sv,D]
  cross_text   = attn.T @ vision    [B,St,D]

Sharding: pure data-parallel over batch, 2 items per core, no collectives.

Design (v2 — PE-stream-first):
  - Host stages activations in BOTH orientations as bf16 (visT/txtT [d,s]
    and vis natural [s,d]) plus weights pre-tiled for direct stationary
    loads.  This removes all on-chip PE transposes of the inputs (256 per
    core in v1) and the staging copies that came with them.
  - All matmul operands are bf16 (PSUM accumulation stays fp32).  At
    N=512 the PE streams 1 row/cycle for bf16 and fp32r alike, so this
    costs nothing on the matmul stream but makes everything resident in
    SBUF (181KB/partition), halves DMA, speeds the E-transposes
    (1.0 vs 1.5 cyc/row) and lets LDWEIGHTS use fast-weight-load.
  - Per item: projK -> projV (V computed directly in [t,d] layout with
    text tiles stationary: no transposes) -> projQ -> F (scores/exp/
    E-transpose/cross_vision, software-pipelined one s-tile ahead) ->
    H (cross_text, 16 sequential single-bank PSUM groups).
  - The 1/sqrt(D) is folded into wqt/bq on the host.  bv is added on the
    host after gather (attn rows sum to 1, so attn @ (v0+bv) = attn@v0
    + bv, exact).
  - Weights are loaded once and stay resident across both items; the
    next item's activations prefetch during the current item's F/H so
    the PE never waits at an item boundary (keeps the PE HAM throttle
    at 8/8 — every >3.4us PE idle gap re-throttles the clock to 1.2GHz
    for ~10us).
"""

import sys

import numpy as np

if "/opt/trn_rl_repo" not in sys.path:
    sys.path.insert(0, "/opt/trn_rl_repo")

import ml_dtypes

import concourse.bass as bass
import concourse.tile as tile
from concourse import bacc
from concourse import mybir

P = 128
B, SEQ, DIM = 16, 1024, 1024
N_CORES = 8
BPC = B // N_CORES  # batch items per core
NT = DIM // P  # 8 tiles of 128 along d/e
F32 = mybir.dt.float32
BF16 = mybir.dt.bfloat16
AF = mybir.ActivationFunctionType
H = 512  # half of a seq dim / PSUM-bank-sized chunk
BF_NP = ml_dtypes.bfloat16


def _emit_item(nc, b, tens, pools, cur, nxt):
    """Emit one batch item.  `cur` holds this item's already-loading
    activation tiles (txtT/visT/visn); prefetch tiles for item b+1 are
    allocated here mid-item and returned via `nxt`."""
    (txtT_d, visT_d, visn_d, cv_d, ct_d) = tens["dram"]
    (wq_sb, wk_sb, wv_sb, bq_sb, bk_sb, ident) = tens["wt"]
    (p_txtT, p_visT, p_visn, p_kt, p_qt, p_vv, p_esb, p_etb, p_cvs, p_cts,
     p_rp, p_rv, pp_t, pp_mm) = pools

    txtT, visT, visn = cur["txtT"], cur["visT"], cur["visn"]
    kt = p_kt.tile([P, NT, SEQ], BF16, name="kt", tag="kt")
    qt = p_qt.tile([P, NT, SEQ], BF16, name="qt", tag="qt")
    vv = p_vv.tile([P, NT, SEQ], BF16, name="vv", tag="vv")
    e_sb = p_esb.tile([P, NT, SEQ], BF16, name="e_sb", tag="esb")

    def nxt_loads(phase):
        """Prefetch item b+1 activations once item b is done reading."""
        if b + 1 >= BPC:
            return
        if phase == "projQ":
            # txtT(b) is dead after projV(b); same buffer, sync queue.
            t2 = p_txtT.tile([P, NT, SEQ], BF16, name="txtT", tag="txtT")
            for l in range(NT):
                nc.sync.dma_start(out=t2[:, l, :], in_=txtT_d[b + 1, :, l, :])
            nxt["txtT"] = t2
        elif phase == "F":
            # visT(b) is dead after projQ(b).
            v2 = p_visT.tile([P, NT, SEQ], BF16, name="visT", tag="visT")
            for l in range(NT):
                nc.sync.dma_start(out=v2[:, l, :], in_=visT_d[b + 1, :, l, :])
            nxt["visT"] = v2
            # vis_nat has bufs=2, so this never blocks the gpsimd queue.
            v3 = p_visn.tile([P, NT, SEQ], BF16, name="visn", tag="visn")
            nc.gpsimd.dma_start(out=v3, in_=visn_d[b + 1])
            nxt["visn"] = v3

    # ---- projK: kt[e, t] = sum_do wkt[do,:,e].T @ txtT[do,:,t] (+bk) ----
    def proj(w_sb, bias_col, xT, out_sb):
        for eo in range(NT):
            pss = [pp_mm.tile([P, H], F32, name=f"ps_p{i}", tag="mm")
                   for i in range(2)]
            for do in range(NT):
                for sh in range(2):
                    nc.tensor.matmul(pss[sh], w_sb[:, eo, do, :],
                                     xT[:, do, sh * H:(sh + 1) * H],
                                     start=(do == 0), stop=(do == NT - 1))
            for sh in range(2):
                dst = out_sb[:, eo, sh * H:(sh + 1) * H]
                if sh == 0:
                    nc.vector.tensor_scalar_add(dst, pss[sh],
                                                scalar1=bias_col[:, eo:eo + 1])
                else:
                    nc.scalar.add(dst, pss[sh], add=bias_col[:, eo:eo + 1])

    proj(wk_sb, bk_sb, txtT, kt)

    # ---- projV: vv[t, d] = sum_do txtT[do,:,t-block].T @ wvt[do,:,d] ----
    for tb in range(NT):
        pss = [pp_mm.tile([P, H], F32, name=f"ps_v{i}", tag="mm")
               for i in range(2)]
        for do in range(NT):
            for dh in range(2):
                nc.tensor.matmul(pss[dh], txtT[:, do, tb * P:(tb + 1) * P],
                                 wv_sb[:, do, dh * H:(dh + 1) * H],
                                 start=(do == 0), stop=(do == NT - 1))
        for dh in range(2):
            dst = vv[:, tb, dh * H:(dh + 1) * H]
            if dh == 0:
                nc.vector.tensor_copy(dst, pss[dh])
            else:
                nc.scalar.copy(dst, pss[dh])

    # ---- projQ ----
    nxt_loads("projQ")
    proj(wq_sb, bq_sb, visT, qt)

    # ---- phase F: scores, softmax, cross_vision (per s-tile) ----
    # Software-pipelined: the scores matmuls of s-tile so+1 are emitted
    # between exp(so) (ACT) and the E-transposes that consume it, so the
    # in-order PE never waits on the ACT engine.
    nxt_loads("F")
    rinv = p_rv.tile([P, NT], F32, name="rinv", tag="rinv")
    rps = {}

    def scores_stile(so):
        rp = p_rp.tile([P, 2], F32, name="rp", tag="rp")
        pss = [pp_mm.tile([P, H], F32, name=f"ps_s{i}", tag="mm")
               for i in range(2)]
        for eo in range(NT):
            for th in range(2):
                nc.tensor.matmul(pss[th], qt[:, eo, so * P:(so + 1) * P],
                                 kt[:, eo, th * H:(th + 1) * H],
                                 start=(eo == 0), stop=(eo == NT - 1))
        for th in range(2):
            nc.scalar.activation(out=e_sb[:, so, th * H:(th + 1) * H],
                                 in_=pss[th], func=AF.Exp,
                                 accum_out=rp[:, th:th + 1])
        rps[so] = rp

    def cross_vision(so, etb):
        # cross_vision[s-tile] = rinv * (ET.T @ Vv)
        cvs = p_cvs.tile([P, DIM], F32, name="cvs", tag="cvs")
        pcv = [pp_mm.tile([P, H], F32, name=f"ps_cv{i}", tag="mm")
               for i in range(2)]
        for tt in range(NT):
            for dh in range(2):
                nc.tensor.matmul(pcv[dh], etb[:, tt, :],
                                 vv[:, tt, dh * H:(dh + 1) * H],
                                 start=(tt == 0), stop=(tt == NT - 1))
        for dh in range(2):
            nc.scalar.mul(cvs[:, dh * H:(dh + 1) * H], pcv[dh],
                          mul=rinv[:, so:so + 1])
        nc.scalar.dma_start(out=cv_d[b, so * P:(so + 1) * P, :], in_=cvs)

    # Pipelined one s-tile deep: iteration `so` emits scores(so+1) between
    # exp(so) (ACT) and the E-transposes that consume it.  CV(so) stays in
    # the same iteration: the short (~0.5us) PE wait on the etb copies is
    # deliberate — a fully airtight PE stream trips the chip's P0 power
    # downclock (2.4 -> 2.0 GHz), which costs far more than the bubbles.
    scores_stile(0)
    for so in range(NT):
        if so + 1 < NT:
            scores_stile(so + 1)
        rp = rps.pop(so)
        rsum = p_rp.tile([P, 1], F32, name="rsum", tag="rsum")
        nc.vector.tensor_add(rsum, rp[:, 0:1], rp[:, 1:2])
        nc.vector.reciprocal(rinv[:, so:so + 1], rsum)

        # ET blocks for this s-tile (transpose the *unnormalized* E row-block)
        etb = p_etb.tile([P, NT, P], BF16, name="etb", tag="etb")
        for tg in range(2):
            tp4 = pp_t.tile([P, 4, P], BF16, name="tp4e", tag="tp4")
            for j in range(4):
                tt = tg * 4 + j
                nc.tensor.matmul(tp4[:, j, :], e_sb[:, so, tt * P:(tt + 1) * P],
                                 ident, is_transpose=True,
                                 start=(j == 0), stop=(j == 3),
                                 skip_group_check=True)
            nc.vector.tensor_copy(etb[:, tg * 4:(tg + 1) * 4, :], tp4)

        # normalize this E row-block in place (for cross_text later)
        nc.vector.tensor_scalar_mul(e_sb[:, so, :], e_sb[:, so, :],
                                    scalar1=rinv[:, so:so + 1])

        cross_vision(so, etb)

    # ---- phase H: cross_text = E'.T @ vis (E' already rinv-scaled) ----
    # 16 sequential single-bank PSUM groups; vis natural is SBUF-resident
    # so the 128 matmuls stream back-to-back with no DMA dependence.
    for dh in range(2):
        for tt in range(NT):
            ps = pp_mm.tile([P, H], F32, name="ps_ct", tag="mm")
            for so in range(NT):
                nc.tensor.matmul(ps, e_sb[:, so, tt * P:(tt + 1) * P],
                                 visn[:, so, dh * H:(dh + 1) * H],
                                 start=(so == 0), stop=(so == NT - 1))
            cts = p_cts.tile([P, H], F32, name="cts", tag="cts")
            if tt % 2 == 0:
                nc.vector.tensor_copy(cts, ps)
            else:
                nc.scalar.copy(cts, ps)
            nc.gpsimd.dma_start(
                out=ct_d[b, tt * P:(tt + 1) * P, dh * H:(dh + 1) * H], in_=cts)


def build_nc():
    nc = bacc.Bacc("TRN2", target_bir_lowering=False, debug=False,
                   num_devices=N_CORES)
    txtT_d = nc.dram_tensor("txtT", [BPC, P, NT, SEQ], BF16,
                            kind="ExternalInput").ap()
    visT_d = nc.dram_tensor("visT", [BPC, P, NT, SEQ], BF16,
                            kind="ExternalInput").ap()
    visn_d = nc.dram_tensor("visn", [BPC, P, NT, SEQ], BF16,
                            kind="ExternalInput").ap()
    wq_d = nc.dram_tensor("wq", [P, NT, NT, P], BF16, kind="ExternalInput").ap()
    wk_d = nc.dram_tensor("wk", [P, NT, NT, P], BF16, kind="ExternalInput").ap()
    wv_d = nc.dram_tensor("wv", [P, NT, SEQ], BF16, kind="ExternalInput").ap()
    bq_d = nc.dram_tensor("bq", [P, NT], F32, kind="ExternalInput").ap()
    bk_d = nc.dram_tensor("bk", [P, NT], F32, kind="ExternalInput").ap()
    id_d = nc.dram_tensor("ident128", [P, P], BF16, kind="ExternalInput").ap()
    cv_d = nc.dram_tensor("cross_vision", [BPC, SEQ, DIM], F32,
                          kind="ExternalOutput").ap()
    ct_d = nc.dram_tensor("cross_text", [BPC, SEQ, DIM], F32,
                          kind="ExternalOutput").ap()

    with tile.TileContext(nc) as tc:
        import contextlib
        with contextlib.ExitStack() as ctx:
            def sp(name, bufs):
                return ctx.enter_context(tc.tile_pool(name=name, bufs=bufs))

            p_wt = sp("wt", 1)
            p_txtT = sp("txtT", 1)
            p_visT = sp("visT", 1)
            p_visn = sp("visn", 2)
            p_kt = sp("kt", 1)
            p_qt = sp("qt", 1)
            p_vv = sp("vv", 1)
            p_esb = sp("esb", 1)
            p_etb = sp("etb", 2)
            p_cvs = sp("cvs", 2)
            p_cts = sp("cts", 4)
            p_rp = sp("rp", 4)
            p_rv = sp("rv", 2)
            pp_t = ctx.enter_context(
                tc.tile_pool(name="pp_t", bufs=2, space=bass.MemorySpace.PSUM))
            pp_mm = ctx.enter_context(
                tc.tile_pool(name="pp_mm", bufs=6, space=bass.MemorySpace.PSUM))

            # small constants first on sync (needed at first proj evac)
            ident = p_wt.tile([P, P], BF16, name="ident")
            nc.sync.dma_start(out=ident, in_=id_d)
            bq_sb = p_wt.tile([P, NT], F32, name="bq_sb")
            nc.sync.dma_start(out=bq_sb, in_=bq_d)
            bk_sb = p_wt.tile([P, NT], F32, name="bk_sb")
            nc.sync.dma_start(out=bk_sb, in_=bk_d)

            # resident weights on gpsimd, chunked by eo (projK's first PSUM
            # group needs only the eo=0 chunk), in use order (K,V,Q)
            wk_sb = p_wt.tile([P, NT, NT, P], BF16, name="wk_sb")
            for eo in range(NT):
                nc.gpsimd.dma_start(out=wk_sb[:, eo, :, :], in_=wk_d[:, eo, :, :])
            wv_sb = p_wt.tile([P, NT, SEQ], BF16, name="wv_sb")
            for do in range(NT):
                nc.gpsimd.dma_start(out=wv_sb[:, do, :], in_=wv_d[:, do, :])
            wq_sb = p_wt.tile([P, NT, NT, P], BF16, name="wq_sb")
            for eo in range(NT):
                nc.gpsimd.dma_start(out=wq_sb[:, eo, :, :], in_=wq_d[:, eo, :, :])

            tens = {
                "dram": (txtT_d, visT_d, visn_d, cv_d, ct_d),
                "wt": (wq_sb, wk_sb, wv_sb, bq_sb, bk_sb, ident),
            }
            pools = (p_txtT, p_visT, p_visn, p_kt, p_qt, p_vv, p_esb, p_etb,
                     p_cvs, p_cts, p_rp, p_rv, pp_t, pp_mm)

            # item-0 activation loads (chunked so the first matmuls can
            # start before the full tensors land): txtT split across the
            # sync and scalar queues (it gates projK's first PSUM group),
            # visT behind it on sync, vis natural on gpsimd behind the
            # weights.
            cur = {}
            cur["txtT"] = p_txtT.tile([P, NT, SEQ], BF16, name="txtT",
                                      tag="txtT")
            for l in range(NT):
                eng = nc.sync if l % 2 == 0 else nc.scalar
                eng.dma_start(out=cur["txtT"][:, l, :],
                              in_=txtT_d[0, :, l, :])
            cur["visT"] = p_visT.tile([P, NT, SEQ], BF16, name="visT",
                                      tag="visT")
            for l in range(NT):
                nc.sync.dma_start(out=cur["visT"][:, l, :],
                                  in_=visT_d[0, :, l, :])
            cur["visn"] = p_visn.tile([P, NT, SEQ], BF16, name="visn",
                                      tag="visn")
            nc.gpsimd.dma_start(out=cur["visn"], in_=visn_d[0])

            for b in range(BPC):
                nxt = {}
                _emit_item(nc, b, tens, pools, cur, nxt)
                cur = nxt
    nc.compile()
    return nc


_NC_CACHE = None


def _get_nc():
    global _NC_CACHE
    if _NC_CACHE is None:
        _NC_CACHE = build_nc()
    return _NC_CACHE


def make_in_maps(vision_repr, text_repr, Wq, bq, Wk, bk, Wv, bv):
    s = 1.0 / np.sqrt(np.float32(DIM))

    def wtile(w, scale=None):
        wt = np.asarray(w, np.float32).T
        if scale is not None:
            wt = wt * scale
        # [d, e] -> [di, eo, do, ei]
        return np.ascontiguousarray(
            wt.reshape(NT, P, NT, P).transpose(1, 2, 0, 3)).astype(BF_NP)

    wq_t = wtile(Wq, s)
    wk_t = wtile(Wk)
    wv_t = np.ascontiguousarray(
        np.asarray(Wv, np.float32).T.reshape(NT, P, SEQ).transpose(1, 0, 2)
    ).astype(BF_NP)
    bq_s = np.ascontiguousarray(
        (np.asarray(bq, np.float32) * s).reshape(NT, P).T)
    bk_s = np.ascontiguousarray(np.asarray(bk, np.float32).reshape(NT, P).T)

    vis = np.asarray(vision_repr, np.float32)
    txt = np.asarray(text_repr, np.float32)
    # [b, s, d] -> transposed [b, di, l, s] and natural [b, si, so, d]
    visT = np.ascontiguousarray(
        vis.transpose(0, 2, 1).reshape(B, NT, P, SEQ).transpose(0, 2, 1, 3)
    ).astype(BF_NP)
    txtT = np.ascontiguousarray(
        txt.transpose(0, 2, 1).reshape(B, NT, P, SEQ).transpose(0, 2, 1, 3)
    ).astype(BF_NP)
    visn = np.ascontiguousarray(
        vis.reshape(B, NT, P, DIM).transpose(0, 2, 1, 3)).astype(BF_NP)

    ident = np.eye(P, dtype=BF_NP)
    in_maps = []
    for c in range(N_CORES):
        sl = slice(c * BPC, (c + 1) * BPC)
        in_maps.append({
            "txtT": txtT[sl], "visT": visT[sl], "visn": visn[sl],
            "wq": wq_t, "wk": wk_t, "wv": wv_t,
            "bq": bq_s, "bk": bk_s,
            "ident128": ident,
        })
    return in_maps


def kernel(vision_repr, text_repr, Wq, bq, Wk, bk, Wv, bv):
    from concourse.bass_utils import run_bass_kernel_spmd

    nc = _get_nc()
    in_maps = make_in_maps(vision_repr, text_repr, Wq, bq, Wk, bk, Wv, bv)
    res = run_bass_kernel_spmd(nc, in_maps, list(range(N_CORES))).results
    cv = np.concatenate([r_["cross_vision"] for r_ in res], axis=0)
    ct = np.concatenate([r_["cross_text"] for r_ in res], axis=0)
    cv = cv + np.asarray(bv, np.float32)[None, None, :]
    return cv, ct
